# revision 80
# baseline (speedup 1.0000x reference)
"""Causal multi-head attention block (qkv -> attention -> proj) on 8 TRN2 cores.

Problem: x[2,2048,1024], w_qkv[3072,1024], b_qkv[3072], w_proj[1024,1024],
b_proj[1024]; H=16 heads, D=64; softmax scale 1/sqrt(1024).

Sharding: core = (batch b, head-group hg); 2 batches x 4 groups of 4 heads.
Each core computes qkv for its 4 heads, causal attention, and a partial
projection (its heads' columns of w_proj); host sums the 4 partials per batch
and adds b_proj.

The PE on this part p-state-ramps: it only reaches full clock (213ns per
512-row pass) after ~3us of UNINTERRUPTED execution; any dependency stall
drops it back to ~1.2GHz (379ns/pass). So the kernel is structured to keep
the PE stream gap-free:
 - slab software pipeline: QK of slab i+1 is emitted BEFORE the PV of slab
   i, so PV's wait on the scalar engine's exp(i) is covered by QK(i+1);
 - phase skew: qkv(c+1) is emitted before proj(c), so proj never waits on
   the DVE normalize and the attention->proj handoff is covered by ~18us of
   qkv matmuls;
 - V reaches its [t,m] layout via XBAR transpose DMAs (sbuf->sbuf), not PE
   transposes, removing both PE passes and DVE drain stalls.

Everything the PE contracts over lives partition-major: x is fed as xT[c,t];
weights are fed pre-transposed. S^T[s,t] = k^T.T @ q^T is computed directly
(no transposes in the S/P path); QK^T packs two heads in the PE via row
tiling (K=64). exp is applied unnormalized and V is augmented with 64 ones
columns so the PV matmul also yields the softmax denominator replicated
across partitions 64..127 (the replication is free: PE pass cost depends on
the moving size N only, not the stationary width). Causality: above-diagonal
s-tiles are skipped; for t-chunks past the first, the diagonal 512x512
square is processed first and narrowed per 128-row s-tile (QK matmul N, exp
width, and PV matmul N all shrink to the causally live t-range; only the
128x128 diagonal block needs a 0/1 triangle mask).

All matmul operands are bf16 (fp16/fp32r measure ~2x/3x slower on this
part; fp8 adds >2% error -- attention outputs are incoherent sums, so
quantization noise does NOT average down). PSUM accumulation is fp32.
y is stored as fp16 and summed on host in fp32.
"""

import numpy as np
import ml_dtypes
from contextlib import ExitStack

import concourse.bass as bass
import concourse.bacc as bacc
import concourse.tile as tile
import concourse.mybir as mybir
from concourse.bass_utils import run_bass_kernel_spmd

B, T, C, H = 2, 2048, 1024, 16
D = C // H                  # 64, head dim
HPC = 4                     # heads per core
N_CORES = 8
NT = T // 128               # 16 t-tiles / s-tiles of 128
NCT = C // 128              # 8 contraction tiles over C
TCH = T // 512              # 4 t-chunks of 512
SCALE = 1.0 / np.sqrt(np.float32(C))   # 1/32

F32 = mybir.dt.float32
F16 = mybir.dt.float16
BF16 = mybir.dt.bfloat16
EXP = mybir.ActivationFunctionType.Exp

VW = 2 * D                  # 128: per-head block in v_sb = [v_h (64) | ones (64)]

_CACHE = {}


def _build():
    """Build + compile the SPMD program (identical on all 8 cores)."""
    nc = bacc.Bacc("TRN2", target_bir_lowering=False, debug=False)

    xT = nc.dram_tensor("xT", [C, T], BF16, kind="ExternalInput")          # x[b].T
    wqkvT = nc.dram_tensor("wqkvT", [C, 3 * HPC * D], BF16, kind="ExternalInput")
    wpT = nc.dram_tensor("wpT", [HPC * D, C], BF16, kind="ExternalInput")
    bqkv = nc.dram_tensor("bqkv", [128, 6], F32, kind="ExternalInput")    # per m-tile
    bvrep = nc.dram_tensor("bvrep", [128, 256], F32, kind="ExternalInput")
    mask = nc.dram_tensor("mask", [128, 2048], BF16, kind="ExternalInput")  # 4x[128,512]
    y = nc.dram_tensor("y", [T, C], F16, kind="ExternalOutput")

    with tile.TileContext(nc) as tc, ExitStack() as ctx:
        sb = ctx.enter_context(tc.tile_pool(name="persist", bufs=1))

        # ---- persistent SBUF tensors ----
        wqkv_sb = sb.tile([128, NCT * 768], BF16, tag="wqkv")      # [c-tile][m 768]
        wp_sb = sb.tile([128, 2 * C], BF16, tag="wp")              # [ci-tile][co 1024]
        bqkv_sb = sb.tile([128, 6], F32, tag="bqkv")
        bv_sb = sb.tile([128, 256], F32, tag="bv")    # b_v replicated over t
        mask_sb = sb.tile([128, 2048], BF16, tag="mask")
        qk_sb = sb.tile([128, 6 * T], BF16, tag="qk")   # q^T|k^T|v^T [m-tile][t]
        v_sb = sb.tile([128, NT * HPC * VW], BF16, tag="v")  # [s-tile][h][v|ones]
        on_sb = sb.tile([128, 2 * T], BF16, tag="onorm")     # O_norm^T [ci-tile][t]

        # ones columns of v_sb (softmax denominator trick), cols 64..127/head
        vdst = v_sb[:].rearrange("p (s h e) -> p s h e", s=NT, h=HPC)[:, :, :, D:VW]
        nc.gpsimd.memset(vdst, 1.0)

        # ---- chunk-pipelined: qkv(0) attn(0) qkv(1) proj(0) attn(1) ... ----
        # PSUM budget (8 banks): sG [128,1024] x2 bufs = 4, acc0+acc1 = 2,
        # shared ps1 pool (qkv accum / proj out) x2 = 2.
        with tc.tile_pool(name="xTp", bufs=1) as xtp, \
             tc.tile_pool(name="ps1", bufs=2, space="PSUM") as ps1, \
             tc.tile_pool(name="ps2", bufs=2, space="PSUM") as ps2, \
             tc.tile_pool(name="psacc", bufs=1, space="PSUM") as psacc, \
             tc.tile_pool(name="att", bufs=4) as att, \
             tc.tile_pool(name="yst", bufs=4) as yst:
            xT_sb = xtp.tile([128, NCT * T], BF16, tag="xT")       # [c-tile][t]
            # chunk-ordered input stream: (w[ct], x[ct, chunk0]) pairs first so
            # the first qkv chain starts after ~2.6MB, then the x tail, then wp
            # (first needed only at proj of chunk 0).
            # w on the SP queue, x on the (still-idle) ACT queue: the two
            # HWDGE queues issue descriptors in parallel, halving the ~600ns
            # per-issue serialization ahead of the first qkv chain. bv/mask
            # ride the ACT queue after the critical wave (first consumers
            # are ~16us in).
            # merged 3D-AP input waves: one dma_start (~600ns issue) per wave
            # instead of one per c-tile, so the ramp is byte- not issue-bound.
            # w wave 1 carries only the q/k m-tile columns.
            w3d = wqkv_sb[:].rearrange("p (c m) -> p c m", c=NCT)
            w3s = wqkvT.ap().rearrange("(c p) m -> p c m", c=NCT)
            x3d = xT_sb[:].rearrange("p (c t) -> p c t", c=NCT)
            x3s = xT.ap().rearrange("(c p) t -> p c t", c=NCT)
            half = NCT // 2
            nc.sync.dma_start(w3d[:, 0:half, 0:384], w3s[:, 0:half, 0:384])
            nc.scalar.dma_start(x3d[:, 0:half, 0:512], x3s[:, 0:half, 0:512])
            nc.sync.dma_start(w3d[:, half:, 0:384], w3s[:, half:, 0:384])
            nc.scalar.dma_start(x3d[:, half:, 0:512], x3s[:, half:, 0:512])
            # bqkv deferred off the critical first wave (first bias ~15us in)
            nc.sync.dma_start(bqkv_sb[:], bqkv.ap())
            nc.sync.dma_start(w3d[:, :, 384:768], w3s[:, :, 384:768])
            nc.scalar.dma_start(bv_sb[:], bvrep.ap())
            nc.scalar.dma_start(mask_sb[:], mask.ap())
            nc.sync.dma_start(x3d[:, :, 512:T], x3s[:, :, 512:T])
            for kt in range(2):
                nc.sync.dma_start(wp_sb[:, kt * C:(kt + 1) * C], wpT.ap()[kt * 128:(kt + 1) * 128, :])

            def qk_chain(tch, mt):
                # one q/k m-tile for t-chunk tch ([m,t] layout; 0,1=q 2,3=k)
                acc = ps1.tile([128, 512], F32, tag="qkacc")
                for ct in range(NCT):
                    nc.tensor.matmul(
                        acc[:],
                        wqkv_sb[:, ct * 768 + mt * 128: ct * 768 + (mt + 1) * 128],
                        xT_sb[:, ct * T + tch * 512: ct * T + tch * 512 + 512],
                        start=(ct == 0), stop=(ct == NCT - 1),
                    )
                nc.vector.tensor_scalar_add(
                    qk_sb[:, mt * T + tch * 512: mt * T + tch * 512 + 512],
                    acc[:], bqkv_sb[:, mt:mt + 1],
                )

            def v_chain(tch, st):
                # one v s-tile directly in [t,m] layout: stationary = x tile
                # [c,t-tile], moving = w_v [c, 256]. No transposes.
                acc = ps1.tile([128, 512], F32, tag="qkacc")
                for ct in range(NCT):
                    nc.tensor.matmul(
                        acc[:, 0:256],
                        xT_sb[:, ct * T + st * 128: ct * T + st * 128 + 128],
                        wqkv_sb[:, ct * 768 + 512: ct * 768 + 768],
                        start=(ct == 0), stop=(ct == NCT - 1),
                    )
                # bias + scatter into the per-head [v 64 | ones 64] blocks
                dst = v_sb[:, st * HPC * VW: (st + 1) * HPC * VW].rearrange(
                    "p (h e) -> p h e", h=HPC)[:, :, 0:D]
                src = acc[:, 0:256].rearrange("p (h e) -> p h e", h=HPC)
                bv = bv_sb[:].rearrange("p (h e) -> p h e", h=HPC)
                nc.vector.tensor_add(dst, src, bv)

            def proj_tt(tch, tt):
                # proj of one t-tile (needs all 4 heads' O_norm at these t)
                for cc in range(2):
                    acc = ps1.tile([128, 512], F32, tag="qkacc")
                    for kt in range(2):
                        nc.tensor.matmul(
                            acc[:],
                            on_sb[:, kt * T + tt * 128: kt * T + tt * 128 + 128],
                            wp_sb[:, kt * C + cc * 512: kt * C + cc * 512 + 512],
                            start=(kt == 0), stop=(kt == 1),
                        )
                    ytile = yst.tile([128, 512], F16, tag="ytile")
                    nc.vector.tensor_copy(ytile[:], acc[:])
                    nc.sync.dma_start(
                        y.ap()[tt * 128:(tt + 1) * 128, cc * 512:(cc + 1) * 512],
                        ytile[:],
                    )

            def emit_qk(tch, hp, g, narrow):
                """QK^T matmuls of one slab -> fresh sG pair; returns tiles."""
                qoff = hp * T        # q m-tile = hp
                koff = (2 + hp) * T  # k m-tile = 2+hp
                sG0 = ps2.tile([128, 1024], F32, tag="sG")
                sG1 = ps2.tile([128, 1024], F32, tag="sG")
                for sG, half in ((sG0, slice(0, 64)), (sG1, slice(64, 128))):
                    for j in range(2):
                        st = 2 * g + j
                        r = st - 4 * tch if narrow else 0   # live t starts at 128r
                        nc.tensor.matmul(
                            sG[:, j * 512 + 128 * r:(j + 1) * 512],
                            qk_sb[half, koff + st * 128: koff + st * 128 + 128],
                            qk_sb[half, qoff + tch * 512 + 128 * r: qoff + tch * 512 + 512],
                            start=True, stop=True,
                            tile_position=(half.start, 0),
                        )
                return sG0, sG1

            def emit_pv(tch, hp, g, narrow, diag, sG0, sG1, acc0, acc1, first, last):
                """exp -> mask -> PV for one slab (consumes the sG pair)."""
                p0 = att.tile([128, 1024], BF16, tag="p0")
                p1 = att.tile([128, 1024], BF16, tag="p1")
                # exp; for narrowed slabs one instr covers the union
                # [128*r0, 1024) -- the dead gap [512, 512+128*r1) holds
                # bounded stale scores (finite under exp) and is never
                # consumed by the narrowed PV passes.
                r0 = 2 * (g - 2 * tch) if narrow else 0
                nc.scalar.activation(p0[:, 128 * r0:1024], sG0[:, 128 * r0:1024], EXP, scale=float(SCALE))
                nc.scalar.activation(p1[:, 128 * r0:1024], sG1[:, 128 * r0:1024], EXP, scale=float(SCALE))
                if narrow:
                    # causal 0/1 triangle on the 128x128 diagonal block
                    for j in range(2):
                        r = r0 + j
                        c0 = j * 512 + 128 * r
                        tri = mask_sb[:, r * 512 + 128 * r: r * 512 + 128 * r + 128]
                        nc.vector.tensor_mul(p0[:, c0:c0 + 128], p0[:, c0:c0 + 128], tri)
                        nc.vector.tensor_mul(p1[:, c0:c0 + 128], p1[:, c0:c0 + 128], tri)
                elif diag:       # tch 0: full-width 0/1 masks
                    mi = (g - 2 * tch) * 1024
                    m = mask_sb[:, mi:mi + 1024]
                    nc.vector.tensor_mul(p0[:], p0[:], m)
                    nc.vector.tensor_mul(p1[:], p1[:], m)
                for acc, p, ho in ((acc0, p0, 0), (acc1, p1, 1)):
                    for j in range(2):
                        st = 2 * g + j
                        r = st - 4 * tch if narrow else 0
                        hc = st * HPC * VW + (2 * hp + ho) * VW
                        nc.tensor.matmul(
                            acc[:, 128 * r:512],
                            v_sb[:, hc:hc + VW],
                            p[:, j * 512 + 128 * r:(j + 1) * 512],
                            start=(first and j == 0), stop=(last and j == 1),
                            # tch0 is all-diagonal: its last (narrowed) PV
                            # can't cover the full acc range, so the sim's
                            # group check must be bypassed; hw correctness
                            # comes from start flags + AP-overlap deps.
                            skip_group_check=(tch == 0),
                        )

            def attn_phase(tch, fillers=(), post_hp0=None):
                # slab g covers s-tiles (2g, 2g+1); diagonal slabs first (the
                # r=0 s-tile opens the full PSUM accumulation range), then the
                # off-diagonal slabs with the last one closing the group.
                if tch == 0:
                    slabs = [(0, True), (1, True)]
                else:
                    # one full-width off-diagonal slab first (opens the PSUM
                    # range and buys time for the v transpose DMAs), then the
                    # narrowed diagonal slabs, then the rest (last one full-
                    # width, closing the accumulation group).
                    slabs = [(0, False), (2 * tch, True), (2 * tch + 1, True)] + \
                            [(g, False) for g in range(1, 2 * tch)]
                n = len(slabs)

                def emit_norm(acc, a, tch=tch):
                    # normalize: O_norm^T = O^T*(1/l), l on rows 64..127.
                    # full-tile recip: the custom-DVE op mishandles
                    # partition slices; rows 0..63 are garbage, unused
                    rl = att.tile([128, 512], F32, tag="rl")
                    nc.vector.reciprocal_approx_fast(rl[:], acc[:])
                    po = (a % 2) * 64
                    dst = on_sb[po:po + 64,
                                (a // 2) * T + tch * 512:(a // 2) * T + tch * 512 + 512]
                    nc.vector.tensor_mul(dst, acc[0:D, :], rl[64:128, :])

                # next-phase PE work (qkv chains of chunk tch+1, proj t-tiles
                # of chunk tch-1) is interleaved between slabs: the scalar
                # engine's exp stream saturates during attention, so these
                # fillers keep the PE fed instead of idling behind exp.
                fillers = list(fillers)
                slots = 2 * n
                per_slot = (len(fillers) + slots - 1) // max(slots, 1)
                for hp in range(2):      # head pair (heads 2hp, 2hp+1)
                    acc0 = psacc.tile([128, 512], F32, tag="acc0")
                    acc1 = psacc.tile([128, 512], F32, tag="acc1")
                    # 1-slab software pipeline: QK(i+1) is on the PE queue
                    # before PV(i), covering PV's wait on exp(i).
                    sgs = emit_qk(tch, hp, slabs[0][0], slabs[0][1])
                    for si, (g, diag) in enumerate(slabs):
                        narrow = diag
                        cur = sgs
                        if si + 1 < n:
                            g2, d2 = slabs[si + 1]
                            sgs = emit_qk(tch, hp, g2, d2)
                        # filler goes BETWEEN QK(i+1) and PV(i): by the time
                        # the PE reaches PV, exp(i) has finished, so the PE
                        # never parks on a semaphore and the DVFS ramp to
                        # full clock survives the whole attention window.
                        for _ in range(per_slot):
                            if fillers:
                                fillers.pop(0)()
                        emit_pv(tch, hp, g, narrow, diag, cur[0], cur[1],
                                acc0, acc1, si == 0, si == n - 1)
                    emit_norm(acc0, 2 * hp)
                    emit_norm(acc1, 2 * hp + 1)
                    if hp == 0 and post_hp0 is not None:
                        post_hp0()
                for f in fillers:        # leftovers (shouldn't normally hit)
                    f()

            def qkv_fillers(tch):
                return [lambda mt=mt: qk_chain(tch, mt) for mt in range(4)] + \
                       [lambda st=st: v_chain(tch, st) for st in range(4 * tch, 4 * tch + 4)]

            def proj_fillers(tch):
                return [lambda tt=tt: proj_tt(tch, tt) for tt in range(4 * tch, 4 * tch + 4)]

            for f in qkv_fillers(0):
                f()
            attn_phase(0, qkv_fillers(1))
            attn_phase(1, qkv_fillers(2) + proj_fillers(0))
            # half of qkv(3) rides in attn(3)'s window (which is otherwise
            # exp-bound with idle PE); its v s-tiles 14,15 land before the
            # g=7 diagonal slab consumes them.
            attn_phase(2, [lambda: qk_chain(3, 0), lambda: qk_chain(3, 2),
                           lambda: v_chain(3, 12), lambda: v_chain(3, 13)] + proj_fillers(1))
            # tail split: proj(3) tt=12's kt=0 passes contract only heads
            # 0,1 (hp0's normalize), so they run during attn(3)-hp1 -- whose
            # fillers are all consumed in hp0, leaving both ps1 banks free.
            held = []

            def proj12_kt0():
                for cc in range(2):
                    acc = ps1.tile([128, 512], F32, tag="qkacc")
                    nc.tensor.matmul(
                        acc[:],
                        on_sb[:, 12 * 128:12 * 128 + 128],
                        wp_sb[:, cc * 512:cc * 512 + 512],
                        start=True, stop=False,
                    )
                    held.append(acc)

            attn_phase(3, [lambda: v_chain(3, 14), lambda: v_chain(3, 15),
                           lambda: qk_chain(3, 1), lambda: qk_chain(3, 3)] + proj_fillers(2),
                       post_hp0=proj12_kt0)
            # tt=13/14 kt=0 prestarts in the (now idle) ps2 banks: they run
            # during the DVE's hp1-normalize drain, which otherwise leaves
            # the PE idle ~2us before proj12's kt=1 can start.
            pre = []
            for tt in (13, 14):
                acc = ps2.tile([128, 1024], F32, tag="sG")
                for cc in range(2):
                    nc.tensor.matmul(
                        acc[:, cc * 512:(cc + 1) * 512],
                        on_sb[:, tt * 128:tt * 128 + 128],
                        wp_sb[:, cc * 512:cc * 512 + 512],
                        start=True, stop=False,
                    )
                pre.append(acc)
            for cc, acc in enumerate(held):
                nc.tensor.matmul(
                    acc[:],
                    on_sb[:, T + 12 * 128:T + 12 * 128 + 128],
                    wp_sb[:, C + cc * 512:C + cc * 512 + 512],
                    start=False, stop=True,
                )
                ytile = yst.tile([128, 512], F16, tag="ytile")
                nc.vector.tensor_copy(ytile[:], acc[:])
                nc.sync.dma_start(
                    y.ap()[12 * 128:13 * 128, cc * 512:(cc + 1) * 512],
                    ytile[:],
                )
            for tt, acc in zip((13, 14), pre):
                for cc in range(2):
                    nc.tensor.matmul(
                        acc[:, cc * 512:(cc + 1) * 512],
                        on_sb[:, T + tt * 128:T + tt * 128 + 128],
                        wp_sb[:, C + cc * 512:C + cc * 512 + 512],
                        start=False, stop=True,
                    )
                for cc in range(2):
                    ytile = yst.tile([128, 512], F16, tag="ytile")
                    nc.vector.tensor_copy(ytile[:], acc[:, cc * 512:(cc + 1) * 512])
                    nc.sync.dma_start(
                        y.ap()[tt * 128:(tt + 1) * 128, cc * 512:(cc + 1) * 512],
                        ytile[:],
                    )
            proj_tt(3, 15)

    nc.compile()
    return nc


def _causal_masks():
    """mask[p, r*512 + j] = 1.0 if (128*r + p) <= j else 0.0, r in 0..3."""
    p = np.arange(128)[:, None]
    j = np.arange(512)[None, :]
    cols = [((128 * r + p) <= j).astype(np.float32) for r in range(4)]
    return np.concatenate(cols, axis=1)


def _in_maps(x, w_qkv, b_qkv, w_proj):
    mask = _causal_masks()
    bf16 = ml_dtypes.bfloat16
    maps = []
    for core in range(N_CORES):
        b, hg = divmod(core, 4)
        h0 = hg * HPC                       # first global head of this core
        r0 = h0 * D                         # first q row
        q_w = w_qkv[r0:r0 + HPC * D]                    # [256, C]
        k_w = w_qkv[C + r0:C + r0 + HPC * D]
        v_w = w_qkv[2 * C + r0:2 * C + r0 + HPC * D]
        wqkvT = np.ascontiguousarray(np.concatenate([q_w, k_w, v_w], axis=0).T)
        wpT = np.ascontiguousarray(w_proj[:, r0:r0 + HPC * D].T)    # [256, C]
        bqkv = np.ascontiguousarray(np.concatenate(
            [b_qkv[r0:r0 + HPC * D], b_qkv[C + r0:C + r0 + HPC * D],
             b_qkv[2 * C + r0:2 * C + r0 + HPC * D]]).reshape(6, 128).T)  # [128,6]
        bvrep = np.ascontiguousarray(np.broadcast_to(
            b_qkv[2 * C + r0:2 * C + r0 + HPC * D][None, :], (128, HPC * D))).astype(np.float32)
        maps.append({
            "xT": np.ascontiguousarray(x[b].T).astype(bf16),
            "wqkvT": wqkvT.astype(bf16),
            "wpT": wpT.astype(bf16),
            "bqkv": bqkv,
            "bvrep": bvrep,
            "mask": mask.astype(bf16),
        })
    return maps


def kernel(x, w_qkv, b_qkv, w_proj, b_proj, _trace=False, _tmpdir=None):
    x = np.asarray(x, dtype=np.float32)
    w_qkv = np.asarray(w_qkv, dtype=np.float32)
    b_qkv = np.asarray(b_qkv, dtype=np.float32)
    w_proj = np.asarray(w_proj, dtype=np.float32)
    b_proj = np.asarray(b_proj, dtype=np.float32)

    if "nc" not in _CACHE:
        _CACHE["nc"] = _build()
    nc = _CACHE["nc"]

    maps = _in_maps(x, w_qkv, b_qkv, w_proj)
    kw = {}
    if _trace:
        kw = {"trace": True, "tmpdir": _tmpdir}
    res = run_bass_kernel_spmd(nc, maps, list(range(N_CORES)), **kw)

    out = np.empty((B, T, C), dtype=np.float32)
    for b in range(B):
        acc = res.results[4 * b]["y"].astype(np.float32)
        for hg in range(1, 4):
            acc = acc + res.results[4 * b + hg]["y"].astype(np.float32)
        out[b] = acc + b_proj[None, :]
    if _trace:
        return out, res
    return out
